# revision 1
# baseline (speedup 1.0000x reference)
"""nn_AutoEncoder (SAE forward) on 8 trn2 NeuronCores.

Strategy (B=4096 sharded 512 rows/core; enc_w/dec replicated):
  Launch 1 (device): project = (embed-bias) @ enc_w.T as float32r matmuls
    (full bf16-rate fp32, ~1.5e-4 abs noise), fused per-512-feature-tile
    top-8 via DVE Max8/MaxIndex, then 10 rounds of max8+match_replace over
    the [128, F/64] tile-max matrix -> per-row top-80 candidate values +
    positions.
  Host: resolve global feature ids; exactly rescore the candidates within
    a +-1.5e-3 band of the rank-64 value (fp32r noise band); re-rank ->
    exact top-64 sets; tiny safety net recomputes any suspicious row.
  Launch 2 (device): decoder -- indirect-DMA gather of the selected
    dec_lookup rows (4 chunk-gathers x 16 vecs x 8 rows per batch),
    weighted-sum via block-diagonal f32r matmuls, +bias.
  total = host bincount of the exact top-64 index sets.
"""
import contextlib
import ctypes
import sys
import types

import numpy as np

# ---------------------------------------------------------------------------
# antenv.axon_hooks shim: lets concourse's trace=True NTFF profiling work
# under axon (used by test harness; harmless otherwise).
if "antenv.axon_hooks" not in sys.modules:
    def _make_hook():
        try:
            lib = ctypes.CDLL("/opt/axon/libaxon_pjrt.so")
        except OSError:
            return None
        if not hasattr(lib, "axon_start_nrt_profile"):
            return None
        lib.axon_start_nrt_profile.argtypes = [
            ctypes.POINTER(ctypes.c_int64), ctypes.c_size_t]
        lib.axon_start_nrt_profile.restype = ctypes.c_int64
        lib.axon_stop_nrt_profile.argtypes = [ctypes.c_char_p]
        lib.axon_stop_nrt_profile.restype = ctypes.c_int64

        @contextlib.contextmanager
        def _hook(output_dir, device_ids):
            import jax
            jax.devices()
            if device_ids:
                ids = (ctypes.c_int64 * len(device_ids))(*device_ids)
                rc = lib.axon_start_nrt_profile(ids, len(device_ids))
            else:
                rc = lib.axon_start_nrt_profile(None, 0)
            if rc != 0:
                raise RuntimeError(f"axon_start_nrt_profile rc={rc}")
            try:
                yield
            finally:
                n = lib.axon_stop_nrt_profile(str(output_dir).encode())
                print(f"[ntff] {n} profile file(s) -> {output_dir}",
                      file=sys.stderr)

        return _hook

    _mod = types.ModuleType("antenv.axon_hooks")
    _hk = _make_hook()
    _mod.get_axon_ntff_profile_hook = lambda: _hk
    _mod.set_axon_ntff_profile_hook = lambda h: None
    sys.modules["antenv.axon_hooks"] = _mod

import concourse.bass as bass
import concourse.mybir as mybir
import concourse.tile as tile
from concourse.bass import IndirectOffsetOnAxis
from concourse.bass_utils import run_bass_kernel_spmd
from concourse.tile import TileContext
from concourse.vector_clock import ScopedClock

F32 = mybir.dt.float32
F32R = mybir.dt.float32r
U32 = mybir.dt.uint32
I32 = mybir.dt.int32
NEG = -1e30

# ---------------------------------------------------------------------------
# walrus compatibility: this toolchain's codegen accepts at most one sem wait
# per instruction; split Tile's multi-wait instructions / tail drain.


def _drain_and_barrier(self, tick_clock, wait_clock):
    nc = self.nc
    carrier = nc.sync.nop(nofuse=True)
    wait_clock.add_sem_waits(carrier.ins,
                            ScopedClock({None: tick_clock.global_clock}))
    si = carrier.ins.sync_info
    if si is not None and si.on_wait and len(si.on_wait) > 1:
        waits = list(si.on_wait)
        si.on_wait = waits[:1]
        for i, w in enumerate(waits[1:]):
            nop = nc.sync.nop(nofuse=True)
            nsi = nop.ins.sync_info
            if nsi is None:
                nop.ins.sync_info = mybir.SyncInfo(on_wait=[w], on_update=[])
            else:
                nsi.on_wait = [w]
    nc.sync.drain()
    nc.all_engine_barrier()
    assert self.sems is not None
    popped = nc._tile_sem_poison_stack.pop()
    assert popped is self._sem_poison
    nc.clear_and_free_semaphores(list(self.sems.allocated().values()))
    nc.all_engine_barrier()


tile.TileContext._drain_and_barrier = _drain_and_barrier


def fix_multiwaits(nc):
    for f in nc.m.functions:
        for b in f.blocks:
            insts = b.instructions
            out = []
            changed = False
            for inst in insts:
                si = inst.sync_info
                if si is not None and len(si.on_wait) > 1:
                    waits = list(si.on_wait)
                    for k, w in enumerate(waits[:-1]):
                        out.append(mybir.InstNoOp(
                            name=f"{inst.name}-waitnop{k}",
                            engine=inst.engine,
                            sync_info=mybir.SyncInfo(on_wait=[w], on_update=[]),
                        ))
                    si.on_wait = [waits[-1]]
                    inst.sync_info = si
                    changed = True
                out.append(inst)
            if changed:
                b.instructions = out


# ---------------------------------------------------------------------------
# kernel builders

def build_launch1(R, E, F, n_rounds=10, wbufs=24):
    KT = E // 128
    FT = F // 512
    G = R // 128
    nc = bass.Bass("TRN2", target_bir_lowering=False, debug=False,
                   num_devices=1)
    xT_d = nc.declare_dram_parameter("xT", [E, R], F32R, isOutput=False)
    wT_d = nc.declare_dram_parameter("wT", [E, F], F32R, isOutput=False)
    V_d = nc.declare_dram_parameter("V", [R, n_rounds * 8], F32, isOutput=True)
    P_d = nc.declare_dram_parameter("P", [R, n_rounds * 8], U32, isOutput=True)
    TI_d = nc.declare_dram_parameter("TI", [R, FT * 8], U32, isOutput=True)

    with TileContext(nc) as tc:
        with (
            tc.tile_pool(name="xpool", bufs=1) as xpool,
            tc.tile_pool(name="wpool", bufs=wbufs) as wpool,
            tc.tile_pool(name="pspool", bufs=2, space="PSUM") as pspool,
            tc.tile_pool(name="tmpool", bufs=1) as tmpool,
        ):
            xts = []
            for g in range(G):
                row = []
                for kt in range(KT):
                    xt = xpool.tile([128, 128], F32R, tag=f"x{g}_{kt}",
                                    name=f"xt{g}_{kt}")
                    nc.sync.dma_start(
                        out=xt,
                        in_=xT_d[kt * 128:(kt + 1) * 128,
                                 g * 128:(g + 1) * 128])
                    row.append(xt)
                xts.append(row)

            TM, TI = [], []
            for g in range(G):
                tm = tmpool.tile([128, FT * 8], F32, tag=f"tm{g}",
                                 name=f"tm{g}")
                ti = tmpool.tile([128, FT * 8], U32, tag=f"ti{g}",
                                 name=f"ti{g}")
                TM.append(tm)
                TI.append(ti)

            for ft in range(FT):
                wts = []
                for kt in range(KT):
                    wt = wpool.tile([128, 512], F32R, tag="wt",
                                    name=f"wt{ft}_{kt}")
                    nc.sync.dma_start(
                        out=wt,
                        in_=wT_d[kt * 128:(kt + 1) * 128,
                                 ft * 512:(ft + 1) * 512])
                    wts.append(wt)
                for g in range(G):
                    ps = pspool.tile([128, 512], F32, tag=f"ps{g % 4}",
                                     name=f"ps{ft}_{g}")
                    for kt in range(KT):
                        nc.tensor.matmul(ps, xts[g][kt], wts[kt],
                                         start=(kt == 0), stop=(kt == KT - 1))
                    sl = slice(ft * 8, (ft + 1) * 8)
                    nc.vector.max(out=TM[g][:, sl], in_=ps)
                    nc.vector.max_index(out=TI[g][:, sl], in_max=TM[g][:, sl],
                                        in_values=ps)

            for g in range(G):
                V = tmpool.tile([128, n_rounds * 8], F32, tag=f"v{g}",
                                name=f"v{g}")
                P = tmpool.tile([128, n_rounds * 8], U32, tag=f"p{g}",
                                name=f"p{g}")
                for r in range(n_rounds):
                    sl = slice(r * 8, (r + 1) * 8)
                    nc.vector.max(out=V[:, sl], in_=TM[g])
                    nc.vector.max_index(out=P[:, sl], in_max=V[:, sl],
                                        in_values=TM[g])
                    nc.vector.match_replace(out=TM[g], in_to_replace=V[:, sl],
                                            in_values=TM[g], imm_value=NEG)
                nc.sync.dma_start(out=V_d[g * 128:(g + 1) * 128, :], in_=V)
                nc.sync.dma_start(out=P_d[g * 128:(g + 1) * 128, :], in_=P)
                nc.sync.dma_start(out=TI_d[g * 128:(g + 1) * 128, :],
                                  in_=TI[g])

    fix_multiwaits(nc)
    return nc


def build_launch2(R, E, F):
    NB = R // 8
    CS = E // 512
    nc = bass.Bass("TRN2", target_bir_lowering=False, debug=False,
                   num_devices=1)
    dec_d = nc.declare_dram_parameter("dec", [F, E], F32R, isOutput=False)
    idx_d = nc.declare_dram_parameter("idx", [128, NB * 4], I32,
                                      isOutput=False)
    lw_d = nc.declare_dram_parameter("lw", [128, NB * 4 * 8], F32R,
                                     isOutput=False)
    bias_d = nc.declare_dram_parameter("biasrow", [8, E], F32, isOutput=False)
    out_d = nc.declare_dram_parameter("embed1", [R, E], F32, isOutput=True)

    with TileContext(nc) as tc:
        with (
            tc.tile_pool(name="const", bufs=1) as cpool,
            tc.tile_pool(name="gp", bufs=2) as gpool,
            tc.tile_pool(name="op", bufs=3) as opool,
            tc.tile_pool(name="psp", bufs=2, space="PSUM") as pspool,
        ):
            idx = cpool.tile([128, NB * 4], I32, name="idx")
            lw = cpool.tile([128, NB * 4 * 8], F32R, name="lw")
            bias = cpool.tile([8, E], F32, name="bias")
            nc.sync.dma_start(out=idx, in_=idx_d[:])
            nc.sync.dma_start(out=lw, in_=lw_d[:])
            nc.sync.dma_start(out=bias, in_=bias_d[:])

            for b in range(NB):
                gs = []
                for c in range(4):
                    gt = gpool.tile([128, E], F32R, tag=f"g{c}",
                                    name=f"g{b}_{c}")
                    nc.gpsimd.indirect_dma_start(
                        out=gt, out_offset=None,
                        in_=dec_d.ap(),
                        in_offset=IndirectOffsetOnAxis(
                            ap=idx[:, b * 4 + c: b * 4 + c + 1], axis=0),
                    )
                    gs.append(gt)
                ot = opool.tile([8, E], F32, tag="ot", name=f"ot{b}")
                for col in range(CS):
                    ps = pspool.tile([8, 512], F32, tag=f"psc{col}",
                                     name=f"ps{b}_{col}")
                    for c in range(4):
                        nc.tensor.matmul(
                            ps,
                            lw[:, (b * 4 + c) * 8:(b * 4 + c + 1) * 8],
                            gs[c][:, col * 512:(col + 1) * 512],
                            start=(c == 0), stop=(c == 3))
                    nc.vector.tensor_add(ot[:, col * 512:(col + 1) * 512], ps,
                                         bias[:, col * 512:(col + 1) * 512])
                nc.sync.dma_start(out=out_d[b * 8:(b + 1) * 8, :], in_=ot)

    fix_multiwaits(nc)
    return nc


_CACHE = {}


def _get_launch(key, builder, *args, **kw):
    if key not in _CACHE:
        _CACHE[key] = builder(*args, **kw)
    return _CACHE[key]


def run_sae(embed, bias, enc_w, dec_lookup, n_cores=8, n_rounds=10,
            band_eps=1.5e-3, verbose=False, trace=False):
    B, E = embed.shape
    F = enc_w.shape[0]
    K = 64
    R = B // n_cores

    x = embed - bias[None, :]
    xT = np.ascontiguousarray(x.T)
    wT = np.ascontiguousarray(enc_w.T)

    nc1 = _get_launch(("l1", R, E, F, n_rounds), build_launch1, R, E, F,
                      n_rounds=n_rounds)
    in_maps = [
        {"xT": np.ascontiguousarray(xT[:, c * R:(c + 1) * R]), "wT": wT}
        for c in range(n_cores)
    ]
    res1 = run_bass_kernel_spmd(nc1, in_maps, core_ids=list(range(n_cores)),
                                trace=trace)
    t1 = res1.exec_time_ns

    V = np.concatenate([res1.results[c]["V"] for c in range(n_cores)], 0)
    P = np.concatenate([res1.results[c]["P"] for c in range(n_cores)], 0)
    TI = np.concatenate([res1.results[c]["TI"] for c in range(n_cores)], 0)

    rows = np.arange(B)[:, None]
    feats = ((P // 8) * 512 + TI[rows, P]).astype(np.int64)
    NR = n_rounds * 8

    bad = np.zeros(B, dtype=bool)
    sf = np.sort(feats, axis=1)
    bad |= (sf[:, 1:] == sf[:, :-1]).any(1)
    st = np.sort(feats // 512, axis=1)
    run = np.ones(B, dtype=np.int64)
    maxrun = np.ones(B, dtype=np.int64)
    for j in range(1, NR):
        run = np.where(st[:, j] == st[:, j - 1], run + 1, 1)
        maxrun = np.maximum(maxrun, run)
    bad |= maxrun >= 8
    v64 = V[:, K - 1]
    bad |= (v64 - V[:, -1]) < band_eps
    bad |= (np.diff(V, axis=1) > 1e-6).any(1)
    bad |= (feats < 0).any(1) | (feats >= F).any(1)

    vals = V.copy()
    band = np.abs(V - v64[:, None]) < band_eps
    band_rows, band_cols = np.nonzero(band & ~bad[:, None])
    if band_rows.size:
        bf = feats[band_rows, band_cols]
        exact = np.einsum("ij,ij->i", x[band_rows], enc_w[bf])
        vals[band_rows, band_cols] = exact
    if verbose:
        print(f"[host] band rescored {band_rows.size} cands, "
              f"bad rows {int(bad.sum())}")

    order = np.argsort(-vals, axis=1, kind="stable")[:, :K]
    top_feats = feats[rows, order]
    top_w = vals[rows, order]

    for r in np.nonzero(bad)[0]:
        pr = x[r] @ enc_w.T
        idx0 = np.argpartition(pr, -K)[-K:]
        idx0 = idx0[np.argsort(-pr[idx0])]
        top_feats[r] = idx0
        top_w[r] = pr[idx0]

    total = np.bincount(top_feats.reshape(-1), minlength=F).astype(np.int32)

    nc2 = _get_launch(("l2", R, E, F), build_launch2, R, E, F)
    NB = R // 8
    j = np.arange(128) // 16
    t = np.arange(128) % 16
    in_maps2 = []
    for c in range(n_cores):
        tf = top_feats[c * R:(c + 1) * R]
        tw = top_w[c * R:(c + 1) * R].astype(np.float32)
        idx_arr = np.empty((128, NB * 4), np.int32)
        lw_arr = np.zeros((128, NB * 4, 8), np.float32)
        for b in range(NB):
            for cc in range(4):
                idx_arr[:, b * 4 + cc] = tf[8 * b + j, 16 * cc + t]
                lw_arr[np.arange(128), b * 4 + cc, j] = tw[8 * b + j,
                                                           16 * cc + t]
        in_maps2.append({
            "dec": dec_lookup,
            "idx": idx_arr,
            "lw": np.ascontiguousarray(lw_arr.reshape(128, NB * 4 * 8)),
            "biasrow": np.ascontiguousarray(
                np.broadcast_to(bias, (8, E))).astype(np.float32),
        })
    res2 = run_bass_kernel_spmd(nc2, in_maps2, core_ids=list(range(n_cores)),
                                trace=trace)
    t2 = res2.exec_time_ns
    embed1 = np.concatenate(
        [res2.results[c]["embed1"] for c in range(n_cores)], 0)

    return embed1, total, (t1, t2)


def kernel(embed, bias, enc_w, dec_lookup):
    embed = np.asarray(embed, dtype=np.float32)
    bias = np.asarray(bias, dtype=np.float32)
    enc_w = np.asarray(enc_w, dtype=np.float32)
    dec_lookup = np.asarray(dec_lookup, dtype=np.float32)
    embed1, total, _ = run_sae(embed, bias, enc_w, dec_lookup, n_cores=8)
    return embed1, total.astype(np.int32)


# revision 2
# speedup vs baseline: 1.3268x; 1.3268x over previous
"""nn_AutoEncoder (SAE forward) on 8 trn2 NeuronCores.

Strategy (B=4096 sharded 512 rows/core; enc_w/dec replicated):
  Launch 1 (device): project = (embed-bias) @ enc_w.T as float32r matmuls
    (full bf16-rate fp32, ~1.5e-4 abs noise), fused per-512-feature-tile
    top-8 via DVE Max8/MaxIndex, then 10 rounds of max8+match_replace over
    the [128, F/64] tile-max matrix -> per-row top-80 candidate values +
    positions.
  Host: resolve global feature ids; exactly rescore the candidates within
    a +-1.5e-3 band of the rank-64 value (fp32r noise band); re-rank ->
    exact top-64 sets; tiny safety net recomputes any suspicious row.
  Launch 2 (device): decoder -- indirect-DMA gather of the selected
    dec_lookup rows (4 chunk-gathers x 16 vecs x 8 rows per batch),
    weighted-sum via block-diagonal f32r matmuls, +bias.
  total = host bincount of the exact top-64 index sets.
"""
import contextlib
import ctypes
import sys
import types

import numpy as np

# ---------------------------------------------------------------------------
# antenv.axon_hooks shim: lets concourse's trace=True NTFF profiling work
# under axon (used by test harness; harmless otherwise).
if "antenv.axon_hooks" not in sys.modules:
    def _make_hook():
        try:
            lib = ctypes.CDLL("/opt/axon/libaxon_pjrt.so")
        except OSError:
            return None
        if not hasattr(lib, "axon_start_nrt_profile"):
            return None
        lib.axon_start_nrt_profile.argtypes = [
            ctypes.POINTER(ctypes.c_int64), ctypes.c_size_t]
        lib.axon_start_nrt_profile.restype = ctypes.c_int64
        lib.axon_stop_nrt_profile.argtypes = [ctypes.c_char_p]
        lib.axon_stop_nrt_profile.restype = ctypes.c_int64

        @contextlib.contextmanager
        def _hook(output_dir, device_ids):
            import jax
            jax.devices()
            if device_ids:
                ids = (ctypes.c_int64 * len(device_ids))(*device_ids)
                rc = lib.axon_start_nrt_profile(ids, len(device_ids))
            else:
                rc = lib.axon_start_nrt_profile(None, 0)
            if rc != 0:
                raise RuntimeError(f"axon_start_nrt_profile rc={rc}")
            try:
                yield
            finally:
                n = lib.axon_stop_nrt_profile(str(output_dir).encode())
                print(f"[ntff] {n} profile file(s) -> {output_dir}",
                      file=sys.stderr)

        return _hook

    _mod = types.ModuleType("antenv.axon_hooks")
    _hk = _make_hook()
    _mod.get_axon_ntff_profile_hook = lambda: _hk
    _mod.set_axon_ntff_profile_hook = lambda h: None
    sys.modules["antenv.axon_hooks"] = _mod

import concourse.bass as bass
import concourse.mybir as mybir
import concourse.tile as tile
from concourse.bass import IndirectOffsetOnAxis
from concourse.bass_utils import run_bass_kernel_spmd
from concourse.tile import TileContext
from concourse.vector_clock import ScopedClock

F32 = mybir.dt.float32
F32R = mybir.dt.float32r
U32 = mybir.dt.uint32
I32 = mybir.dt.int32
NEG = -1e30
F16 = mybir.dt.float16

# ---------------------------------------------------------------------------
# walrus compatibility: this toolchain's codegen accepts at most one sem wait
# per instruction; split Tile's multi-wait instructions / tail drain.


def _drain_and_barrier(self, tick_clock, wait_clock):
    nc = self.nc
    carrier = nc.sync.nop(nofuse=True)
    wait_clock.add_sem_waits(carrier.ins,
                            ScopedClock({None: tick_clock.global_clock}))
    si = carrier.ins.sync_info
    if si is not None and si.on_wait and len(si.on_wait) > 1:
        waits = list(si.on_wait)
        si.on_wait = waits[:1]
        for i, w in enumerate(waits[1:]):
            nop = nc.sync.nop(nofuse=True)
            nsi = nop.ins.sync_info
            if nsi is None:
                nop.ins.sync_info = mybir.SyncInfo(on_wait=[w], on_update=[])
            else:
                nsi.on_wait = [w]
    nc.sync.drain()
    nc.all_engine_barrier()
    assert self.sems is not None
    popped = nc._tile_sem_poison_stack.pop()
    assert popped is self._sem_poison
    nc.clear_and_free_semaphores(list(self.sems.allocated().values()))
    nc.all_engine_barrier()


tile.TileContext._drain_and_barrier = _drain_and_barrier


def fix_multiwaits(nc):
    for f in nc.m.functions:
        for b in f.blocks:
            insts = b.instructions
            out = []
            changed = False
            for inst in insts:
                si = inst.sync_info
                if si is not None and len(si.on_wait) > 1:
                    waits = list(si.on_wait)
                    for k, w in enumerate(waits[:-1]):
                        out.append(mybir.InstNoOp(
                            name=f"{inst.name}-waitnop{k}",
                            engine=inst.engine,
                            sync_info=mybir.SyncInfo(on_wait=[w], on_update=[]),
                        ))
                    si.on_wait = [waits[-1]]
                    inst.sync_info = si
                    changed = True
                out.append(inst)
            if changed:
                b.instructions = out


# ---------------------------------------------------------------------------
# kernel builders

def build_launch1(R, E, F, n_rounds=10, wbufs=32):
    KT = E // 128
    FT = F // 512
    G = R // 128
    nc = bass.Bass("TRN2", target_bir_lowering=False, debug=False,
                   num_devices=1)
    xT_d = nc.declare_dram_parameter("xT", [E, R], F16, isOutput=False)
    wT_d = nc.declare_dram_parameter("wT", [E, F], F16, isOutput=False)
    V_d = nc.declare_dram_parameter("V", [R, n_rounds * 8], F32, isOutput=True)
    P_d = nc.declare_dram_parameter("P", [R, n_rounds * 8], U32, isOutput=True)
    TI_d = nc.declare_dram_parameter("TI", [R, FT * 8], U32, isOutput=True)

    with TileContext(nc) as tc:
        with (
            tc.tile_pool(name="xpool", bufs=1) as xpool,
            tc.tile_pool(name="wpool", bufs=wbufs) as wpool,
            tc.tile_pool(name="pspool", bufs=2, space="PSUM") as pspool,
            tc.tile_pool(name="tmpool", bufs=1) as tmpool,
        ):
            xts = []
            for g in range(G):
                row = []
                for kt in range(KT):
                    xt = xpool.tile([128, 128], F16, tag=f"x{g}_{kt}",
                                    name=f"xt{g}_{kt}")
                    nc.sync.dma_start(
                        out=xt,
                        in_=xT_d[kt * 128:(kt + 1) * 128,
                                 g * 128:(g + 1) * 128])
                    row.append(xt)
                xts.append(row)

            TM, TI = [], []
            for g in range(G):
                tm = tmpool.tile([128, FT * 8], F32, tag=f"tm{g}",
                                 name=f"tm{g}")
                ti = tmpool.tile([128, FT * 8], U32, tag=f"ti{g}",
                                 name=f"ti{g}")
                TM.append(tm)
                TI.append(ti)

            for ft in range(FT):
                wts = []
                for kt in range(KT):
                    wt = wpool.tile([128, 512], F16, tag="wt",
                                    name=f"wt{ft}_{kt}")
                    nc.sync.dma_start(
                        out=wt,
                        in_=wT_d[kt * 128:(kt + 1) * 128,
                                 ft * 512:(ft + 1) * 512])
                    wts.append(wt)
                for g in range(G):
                    ps = pspool.tile([128, 512], F32, tag=f"ps{g % 4}",
                                     name=f"ps{ft}_{g}")
                    for kt in range(KT):
                        nc.tensor.matmul(ps, xts[g][kt], wts[kt],
                                         start=(kt == 0), stop=(kt == KT - 1))
                    sl = slice(ft * 8, (ft + 1) * 8)
                    nc.vector.max(out=TM[g][:, sl], in_=ps)
                    nc.vector.max_index(out=TI[g][:, sl], in_max=TM[g][:, sl],
                                        in_values=ps)

            for g in range(G):
                V = tmpool.tile([128, n_rounds * 8], F32, tag=f"v{g}",
                                name=f"v{g}")
                P = tmpool.tile([128, n_rounds * 8], U32, tag=f"p{g}",
                                name=f"p{g}")
                for r in range(n_rounds):
                    sl = slice(r * 8, (r + 1) * 8)
                    nc.vector.max(out=V[:, sl], in_=TM[g])
                    nc.vector.max_index(out=P[:, sl], in_max=V[:, sl],
                                        in_values=TM[g])
                    nc.vector.match_replace(out=TM[g], in_to_replace=V[:, sl],
                                            in_values=TM[g], imm_value=NEG)
                nc.sync.dma_start(out=V_d[g * 128:(g + 1) * 128, :], in_=V)
                nc.sync.dma_start(out=P_d[g * 128:(g + 1) * 128, :], in_=P)
                nc.sync.dma_start(out=TI_d[g * 128:(g + 1) * 128, :],
                                  in_=TI[g])

    fix_multiwaits(nc)
    return nc


def build_launch2(R, E, F):
    NB = R // 8
    CS = E // 512
    nc = bass.Bass("TRN2", target_bir_lowering=False, debug=False,
                   num_devices=1)
    dec_d = nc.declare_dram_parameter("dec", [F, E], F16, isOutput=False)
    idx_d = nc.declare_dram_parameter("idx", [128, NB * 4], I32,
                                      isOutput=False)
    lw_d = nc.declare_dram_parameter("lw", [128, NB * 4 * 8], F16,
                                     isOutput=False)
    bias_d = nc.declare_dram_parameter("biasrow", [8, E], F32, isOutput=False)
    out_d = nc.declare_dram_parameter("embed1", [R, E], F32, isOutput=True)

    with TileContext(nc) as tc:
        with (
            tc.tile_pool(name="const", bufs=1) as cpool,
            tc.tile_pool(name="gp", bufs=2) as gpool,
            tc.tile_pool(name="op", bufs=3) as opool,
            tc.tile_pool(name="psp", bufs=2, space="PSUM") as pspool,
        ):
            idx = cpool.tile([128, NB * 4], I32, name="idx")
            lw = cpool.tile([128, NB * 4 * 8], F16, name="lw")
            bias = cpool.tile([8, E], F32, name="bias")
            nc.sync.dma_start(out=idx, in_=idx_d[:])
            nc.sync.dma_start(out=lw, in_=lw_d[:])
            nc.sync.dma_start(out=bias, in_=bias_d[:])

            for b in range(NB):
                gs = []
                for c in range(4):
                    gt = gpool.tile([128, E], F16, tag=f"g{c}",
                                    name=f"g{b}_{c}")
                    nc.gpsimd.indirect_dma_start(
                        out=gt, out_offset=None,
                        in_=dec_d.ap(),
                        in_offset=IndirectOffsetOnAxis(
                            ap=idx[:, b * 4 + c: b * 4 + c + 1], axis=0),
                    )
                    gs.append(gt)
                ot = opool.tile([8, E], F32, tag="ot", name=f"ot{b}")
                for col in range(CS):
                    ps = pspool.tile([8, 512], F32, tag=f"psc{col}",
                                     name=f"ps{b}_{col}")
                    for c in range(4):
                        nc.tensor.matmul(
                            ps,
                            lw[:, (b * 4 + c) * 8:(b * 4 + c + 1) * 8],
                            gs[c][:, col * 512:(col + 1) * 512],
                            start=(c == 0), stop=(c == 3))
                    nc.vector.tensor_add(ot[:, col * 512:(col + 1) * 512], ps,
                                         bias[:, col * 512:(col + 1) * 512])
                nc.sync.dma_start(out=out_d[b * 8:(b + 1) * 8, :], in_=ot)

    fix_multiwaits(nc)
    return nc


_CACHE = {}


def _get_launch(key, builder, *args, **kw):
    if key not in _CACHE:
        _CACHE[key] = builder(*args, **kw)
    return _CACHE[key]


def run_sae(embed, bias, enc_w, dec_lookup, n_cores=8, n_rounds=10,
            band_eps=2e-3, verbose=False, trace=False):
    B, E = embed.shape
    F = enc_w.shape[0]
    K = 64
    R = B // n_cores

    x = embed - bias[None, :]
    xT = np.ascontiguousarray(x.T.astype(np.float16))
    wT = np.ascontiguousarray(enc_w.T.astype(np.float16))

    nc1 = _get_launch(("l1", R, E, F, n_rounds), build_launch1, R, E, F,
                      n_rounds=n_rounds)
    in_maps = [
        {"xT": np.ascontiguousarray(xT[:, c * R:(c + 1) * R]), "wT": wT}
        for c in range(n_cores)
    ]
    res1 = run_bass_kernel_spmd(nc1, in_maps, core_ids=list(range(n_cores)),
                                trace=trace)
    t1 = res1.exec_time_ns

    V = np.concatenate([res1.results[c]["V"] for c in range(n_cores)], 0)
    P = np.concatenate([res1.results[c]["P"] for c in range(n_cores)], 0)
    TI = np.concatenate([res1.results[c]["TI"] for c in range(n_cores)], 0)

    P = np.where(P >= TI.shape[1], 0, P)
    rows = np.arange(B)[:, None]
    feats = ((P // 8) * 512 + TI[rows, P]).astype(np.int64)
    NR = n_rounds * 8

    bad = np.zeros(B, dtype=bool)
    bad |= ~np.isfinite(V).all(1)
    sf = np.sort(feats, axis=1)
    bad |= (sf[:, 1:] == sf[:, :-1]).any(1)
    st = np.sort(feats // 512, axis=1)
    run = np.ones(B, dtype=np.int64)
    maxrun = np.ones(B, dtype=np.int64)
    for j in range(1, NR):
        run = np.where(st[:, j] == st[:, j - 1], run + 1, 1)
        maxrun = np.maximum(maxrun, run)
    bad |= maxrun >= 8
    v64 = V[:, K - 1]
    bad |= (v64 - V[:, -1]) < band_eps
    bad |= (np.diff(V, axis=1) > 1e-6).any(1)
    bad |= (feats < 0).any(1) | (feats >= F).any(1)

    vals = V.copy()
    band = np.abs(V - v64[:, None]) < band_eps
    band_rows, band_cols = np.nonzero(band & ~bad[:, None])
    if band_rows.size:
        bf = feats[band_rows, band_cols]
        exact = np.einsum("ij,ij->i", x[band_rows], enc_w[bf])
        vals[band_rows, band_cols] = exact
    if verbose:
        print(f"[host] band rescored {band_rows.size} cands, "
              f"bad rows {int(bad.sum())}")

    order = np.argsort(-vals, axis=1, kind="stable")[:, :K]
    top_feats = feats[rows, order]
    top_w = vals[rows, order]

    for r in np.nonzero(bad)[0]:
        pr = x[r] @ enc_w.T
        idx0 = np.argpartition(pr, -K)[-K:]
        idx0 = idx0[np.argsort(-pr[idx0])]
        top_feats[r] = idx0
        top_w[r] = pr[idx0]

    total = np.bincount(top_feats.reshape(-1), minlength=F).astype(np.int32)

    dec_f16 = dec_lookup.astype(np.float16)
    nc2 = _get_launch(("l2", R, E, F), build_launch2, R, E, F)
    NB = R // 8
    j = np.arange(128) // 16
    t = np.arange(128) % 16
    in_maps2 = []
    for c in range(n_cores):
        tf = top_feats[c * R:(c + 1) * R]
        tw = top_w[c * R:(c + 1) * R].astype(np.float32)
        idx_arr = np.empty((128, NB * 4), np.int32)
        lw_arr = np.zeros((128, NB * 4, 8), np.float32)
        for b in range(NB):
            for cc in range(4):
                idx_arr[:, b * 4 + cc] = tf[8 * b + j, 16 * cc + t]
                lw_arr[np.arange(128), b * 4 + cc, j] = tw[8 * b + j,
                                                           16 * cc + t]
        in_maps2.append({
            "dec": dec_f16,
            "idx": idx_arr,
            "lw": np.ascontiguousarray(lw_arr.reshape(128, NB * 4 * 8).astype(np.float16)),
            "biasrow": np.ascontiguousarray(
                np.broadcast_to(bias, (8, E))).astype(np.float32),
        })
    res2 = run_bass_kernel_spmd(nc2, in_maps2, core_ids=list(range(n_cores)),
                                trace=trace)
    t2 = res2.exec_time_ns
    embed1 = np.concatenate(
        [res2.results[c]["embed1"] for c in range(n_cores)], 0)

    return embed1, total, (t1, t2)


def kernel(embed, bias, enc_w, dec_lookup):
    embed = np.asarray(embed, dtype=np.float32)
    bias = np.asarray(bias, dtype=np.float32)
    enc_w = np.asarray(enc_w, dtype=np.float32)
    dec_lookup = np.asarray(dec_lookup, dtype=np.float32)
    embed1, total, _ = run_sae(embed, bias, enc_w, dec_lookup, n_cores=8)
    return embed1, total.astype(np.int32)
